# revision 136
# baseline (speedup 1.0000x reference)
"""Trainium2 Bass kernel for nn_MiMoV2FlashBlock (GQA attention block with
partial RoPE and attention-sink softmax), sharded across 8 NeuronCores.

Sharding: tensor-parallel over heads. Core i computes q-heads 4i..4i+3 and
kv-head i (H=32, KVH=8 -> each GQA group of 4 q-heads shares exactly one kv
head), plus the matching input-dim shard of the o-projection. Each core emits
a partial [S, HID] output; the host sums the 8 partials.

Precision: the q/k/v projections run as 3-term fp8e4m3 DoubleRow matmuls
(hi/lo splits computed host-side; 256-deep contraction at 0.5 cycles/row =
4x bf16 FLOP rate, so the 3-term decomposition costs 0.75x the bf16
projection while keeping bf16-grade accuracy — the dropped lo*lo term is
~2^-16 relative). Weights are host-scaled by 64 so the fp8 splits sit in
e4m3's normal range; the q/k scale cancels in the exp scale and the v scale
is divided back out of Wo. Scores/PV/o-proj stay bf16: a 128-deep
contraction cannot pair k-tiles for DoubleRow (3-term there costs 1.5x),
single-fp8 anywhere in the value path measures 3-6% error vs the 2e-2 gate,
and an fp8 o-proj requires an on-chip oT hi/lo split whose DVE cost and
dependency chain outweigh the PE savings (measured +3us net). Accumulation
is fp32 in PSUM throughout; end-to-end error is ~3.6e-3 max-rel.

On-core dataflow:
  phase A: qT/kT = W @ hs^T per head ([D,S] layout, fp8 3-term DoubleRow
           over hid-tile pairs); v projects directly in [S,VD] layout
           (hs-tile stationary, wv moving, its four per-strip accumulators
           packed into one PSUM bank with a single start=True). Partial RoPE
           runs entirely off the PE: the rope dims are interleaved host-side
           so rotate-half is a DVE stream_shuffle (sign folded into a signed
           sin table), then sin-mult + cos-add on DVE with the cos-products
           precomputed on Pool.
  phase B (per q-strip j, heads in pairs): scores^T [128k, <=512q] = k @ q^T,
           trimmed per-block to the causally-valid q-column range; exp on ACT
           (PSUM->SBUF bf16); mask blocks multiply by exp(mask) sub-tiles;
           probs accumulate into a per-(h,j) bf16 tile on DVE whose
           cross-partition sum (the softmax denominator, incl. the sink
           term) comes from one gpsimd partition_all_reduce; DVE reciprocal;
           PV accumulates out^T in PSUM; normalize multiplies PSUM by the
           broadcast reciprocal writing bf16 oT.
           Softmax is computed without a max-subtraction pass: logits here
           are O(10), far below fp32/bf16 exp overflow; the host verifies
           the final output is finite.
  phase C: o_partial = oT-shard @ Wo-shard per 128x512 chunk, PSUM -> SBUF
           copy (alternating ACT/DVE) -> per-chunk DMA to DRAM. Each strip's
           chunks are queued once its softmax tails are flushed and then
           interleaved into later strips' attention streams as PE
           bubble-filler; leftovers drain at the end.

Scheduling: phase B processes strips in order [NJ-2, NJ-1, 0, 1, ...] so the
phase boundary starts on a dense strip; the q-head rope of all strips except
NJ-2 is deferred from phase A into the same filler queue (each strip's rope
is forced to emit before that strip's attention).

Causality (or any additive-mask structure) is exploited host-side: fully
masked 512x128 blocks are skipped, each remaining block is trimmed to the
rows with any valid entry, and only sub-ranges with nonzero mask entries
multiply by exp(mask). Exact for arbitrary masks: exp(s+m) = exp(s)*exp(m).
"""

import sys

for _p in ("/opt/trn_rl_repo",):
    if _p not in sys.path:
        sys.path.insert(0, _p)

import numpy as np
import ml_dtypes

import concourse.bass as bass  # noqa: E402
import concourse.mybir as mybir  # noqa: E402
import concourse.tile as tile  # noqa: E402
from concourse import bacc  # noqa: E402
from concourse import bass_isa  # noqa: E402
from concourse.bass_utils import run_bass_kernel_spmd  # noqa: E402

F32 = mybir.dt.float32
F32R = mybir.dt.float32r
BF16 = mybir.dt.bfloat16
FP8 = mybir.dt.float8e4
PM = mybir.MatmulPerfMode
AF = mybir.ActivationFunctionType
ALU = mybir.AluOpType
BF16_NP = ml_dtypes.bfloat16
FP8_NP = ml_dtypes.float8_e4m3fn

# Problem shape (hardcoded per the harness contract).
B, S, HID = 1, 2048, 2048
H, KVH, D, VD = 32, 8, 128, 128
R = 64
N_CORES = 8
QH_L = H // N_CORES          # 4 local q heads per core
SCALE = float(D) ** -0.5

NT = S // 128                # 16 seq tiles of 128
NH = HID // 128              # 16 hidden (contraction) tiles
NJ = S // 512                # 4 q-strips of 512
DT = (QH_L * VD) // 128      # 4 o-proj contraction tiles

# Q/K/V weights are host-scaled by WSCALE (so fp8 hi/lo splits sit in e4m3's
# normal range); q,k pick up WSCALE each -> scores x WSCALE^2, absorbed into
# the exp scale. v picks up WSCALE -> folded into Wo (exact power of two).
WSCALE = 64.0
EXP_SCALE = SCALE / (WSCALE * WSCALE)

_cache: dict = {}


def _pack_pairs(blocks):
    """Pair consecutive blocks and assign bank-safe column offsets inside a
    [128, 1024] scores tile (a single matmul may not cross the 512-col PSUM
    bank boundary). Returns [(blockinfo, off), ...] per pair plus the
    contiguous exp runs [(lo, hi), ...]."""
    pairs = []
    for i in range(0, len(blocks), 2):
        grp = blocks[i:i + 2]
        offs, off = [], 0
        for (kt, qlo, qhi, midx, mlo, mhi) in grp:
            w = qhi - qlo
            if off < 512 and (off + w) > 512:
                off = 512
            offs.append(off)
            off += w
        runs, cur = [], None
        for (kt, qlo, qhi, midx, mlo, mhi), o in zip(grp, offs):
            w = qhi - qlo
            if cur is not None and cur[1] == o:
                cur = (cur[0], o + w)
            else:
                if cur is not None:
                    runs.append(cur)
                cur = (o, o + w)
        runs.append(cur)
        pairs.append((grp, offs, runs))
    return pairs


def _build(schedule, n_masks):
    """Build + compile the per-core SPMD module for a given mask schedule.

    schedule[j][kt] is None (fully masked block) or a tuple
    (qlo, qhi, mask_idx, mlo, mhi): valid q rows [qlo, qhi) of the 512-strip,
    mask_idx >= 0 selects an exp(mask) tile applied on q in [mlo, mhi).
    """
    nc = bacc.Bacc(None, target_bir_lowering=False)

    # hs_il slot order (lo, hi); w*_il slot order (hi, lo). Then per hid-tile
    # pair (t, t+1) the 3-term fp8 DoubleRow decomposition of W @ h is:
    #   main:  lhsT = w[:, t:t+2, 0, :] (hi,hi), rhs = hs[:, t:t+2, 1, :]
    #   cross: lhsT = w[:, t, :, :] (hi,lo),  rhs = hs[:, t, :, :] (lo,hi)
    # i.e. W_hi@h_hi over the pair plus (W_hi@h_lo + W_lo@h_hi) per tile; the
    # dropped W_lo@h_lo term is ~2^-16 relative (below bf16 input rounding).
    hs_h = nc.dram_tensor("hsx", [128, NH, 2, S], FP8, kind="ExternalInput")
    wq_h = nc.dram_tensor("wq", [128, NH, 2, QH_L * 128], FP8, kind="ExternalInput")
    wk_h = nc.dram_tensor("wk", [128, NH, 2, 128], FP8, kind="ExternalInput")
    wv_h = nc.dram_tensor("wv", [128, NH, 2, 128], FP8, kind="ExternalInput")
    wo_h = nc.dram_tensor("wo", [128, DT, HID], BF16, kind="ExternalInput")
    cs_h = nc.dram_tensor("csT", [64, S], BF16, kind="ExternalInput")
    sn_h = nc.dram_tensor("snT", [64, S], BF16, kind="ExternalInput")
    sink_h = nc.dram_tensor("sink128", [128, QH_L], F32, kind="ExternalInput")
    nm = max(n_masks, 1)
    em_h = nc.dram_tensor("emask", [nm, 128, 512], BF16, kind="ExternalInput")
    out_h = nc.dram_tensor("out", [S, HID], BF16, kind="ExternalOutput")

    lp = nc.allow_low_precision(
        reason="bf16 operands; fp32 accumulation; validated 4e-3 vs 2e-2 gate"
    )
    lp.__enter__()

    with tile.TileContext(nc) as tc:
        with (
            tc.tile_pool(name="consts", bufs=1) as cpool,
            tc.tile_pool(name="qkv", bufs=1) as qkvpool,
            tc.tile_pool(name="small", bufs=2) as spool,
            tc.tile_pool(name="probs", bufs=13) as prpool,
            tc.tile_pool(name="psVC", bufs=2, space="PSUM") as psVC,
        ):
            sink128 = cpool.tile([128, QH_L], F32)
            csT = cpool.tile([64, S], BF16)
            snT = cpool.tile([64, S], BF16)

            qT = [qkvpool.tile([128, S], BF16, tag=f"qT{h}", name=f"qT{h}")
                  for h in range(QH_L)]
            kT = qkvpool.tile([128, S], BF16, tag="kT")
            vsb = qkvpool.tile([128, NT, VD], BF16, tag="v")
            # phase B/C long-lived tensors allocated before phase A so their
            # DMAs (idle gpsimd queue) prefetch under phase A's compute and
            # no SBUF-reuse WAR stall appears at the phase boundary
            oT = [qkvpool.tile([128, S], BF16, tag=f"oT{h}", name=f"oT{h}")
                  for h in range(QH_L)]
            wo = qkvpool.tile([128, DT, HID], BF16, tag="wo")
            emask = [qkvpool.tile([128, 512], BF16, tag=f"em{m}",
                                  name=f"em{m}")
                     for m in range(n_masks)]

            # ================= phase A: projections + rope =================
            with (
                tc.tile_pool(name="phA", bufs=1) as apool,
                tc.tile_pool(name="hsxp", bufs=4) as hsxpool,
                tc.tile_pool(name="psA", bufs=6, space="PSUM") as psA,
            ):
                wq = apool.tile([128, NH, 2, QH_L * 128], FP8)
                wk = apool.tile([128, NH, 2, 128], FP8)
                wv = apool.tile([128, NH, 2, 128], FP8)

                ppn = [0]   # pp-tag allocation counter (for slot mapping)

                SWAP_MASK = [i ^ 1 for i in range(32)]

                def rope_head(j, hh, m1, auxpool, auxtag, on_pool=False):
                    """Apply partial rope to one head's [D, 512] strip.
                    The rope dims are interleaved host-side (row 2d = orig d,
                    row 2d+1 = orig d+32) so rotate-half is an adjacent-row
                    swap: one DVE stream_shuffle, sign folded into the signed
                    sin table; then sin-mult + cos-add (the cos-product m1
                    was precomputed on Pool). With on_pool=True the mult/add
                    run on Pool: slower per-op but off DVE, which carries the
                    phase-B mask/acc stream these deferred ropes race with."""
                    jsl = slice(512 * j, 512 * (j + 1))
                    dst = qT[hh] if hh < QH_L else kT
                    eng = nc.gpsimd if on_pool else nc.vector
                    qsw = spool.tile([R, 512], BF16, tag="qsb", bufs=3)
                    nc.vector.stream_shuffle(qsw[:], dst[0:R, jsl], SWAP_MASK)
                    eng.tensor_tensor(
                        dst[0:R, jsl], qsw[:], snT[:, jsl], ALU.mult
                    )
                    eng.tensor_tensor(
                        dst[0:R, jsl], dst[0:R, jsl], m1[:], ALU.add
                    )


                rope_pending = None
                rope_defer = []   # (strip, thunk(auxpool)) consumed in phase B
                for j in range(NJ):
                    jsl = slice(512 * j, 512 * (j + 1))
                    if j == NJ - 1 and rope_pending is not None:
                        # strip NJ-2's full rope leads the last iteration: its
                        # inputs are ready, the PE cost is ~1us, and its
                        # ACT/DVE chain then drains under this strip's
                        # projections so phase B starts rope-free
                        rope_pending()
                        rope_pending = None
                    pp_slot0 = ppn[0]
                    ppn[0] += QH_L + 1
                    pp = [psA.tile([128, 512], F32, tag="pp", name=f"pp{j}_{i}")
                          for i in range(QH_L + 1)]
                    ppv = psVC.tile([128, 512], F32, tag="oc",
                                    name=f"ppv{j}")
                    hx4 = [None] * 4
                    if j == 1:
                        # emask prefetch deferred off the startup-critical
                        # SWDGE queue
                        for m in range(n_masks):
                            nc.gpsimd.dma_start(emask[m][:], em_h[m, :, :])
                    if j == 0:
                        # startup: a micro-slice of the head-0 weights plus
                        # the first hs pair go out first, one per DMA queue,
                        # so the first matmul's operands land after a single
                        # config+transfer; remainders stagger behind
                        hx4[0] = hsxpool.tile([128, 4, 2, 512], FP8, tag="hsx",
                                              name="hsx0_0")
                        # hx via SWDGE (Pool clears its init barrier first,
                        # ~440ns) and wq via SP; only the less-urgent wk/wv
                        # sit behind the ~1.3us act-table load on the ACT
                        # queue's sequencer
                        nc.gpsimd.dma_start(hx4[0][:, 0:2, :, :],
                                            hs_h[:, 0:2, :, jsl])
                        nc.sync.dma_start(wq[:, 0:2], wq_h[:, 0:2])
                        nc.scalar.dma_start(wk[:, 0:2], wk_h[:, 0:2])
                        nc.gpsimd.dma_start(hx4[0][:, 2:4, :, :],
                                            hs_h[:, 2:4, :, jsl])
                        nc.scalar.dma_start(wv[:, 0:2], wv_h[:, 0:2])
                        nc.sync.dma_start(wq[:, 2:4], wq_h[:, 2:4])
                        nc.scalar.dma_start(wk[:, 2:8], wk_h[:, 2:8])
                        nc.scalar.dma_start(wv[:, 2:8], wv_h[:, 2:8])
                        nc.scalar.dma_start(wq[:, 4:8], wq_h[:, 4:8])
                        nc.scalar.dma_start(wq[:, 8:12], wq_h[:, 8:12])
                        nc.scalar.dma_start(wk[:, 8:16], wk_h[:, 8:16])
                        nc.scalar.dma_start(wv[:, 8:16], wv_h[:, 8:16])
                        nc.scalar.dma_start(wq[:, 12:16], wq_h[:, 12:16])
                        nc.scalar.dma_start(csT[:], cs_h[:])
                        nc.scalar.dma_start(snT[:], sn_h[:])
                        nc.scalar.dma_start(sink128[:], sink_h[:])
                    for tp in range(NH // 2):
                        t = 2 * tp
                        g, r = t // 4, t % 4   # r in {0, 2}; t,t+1 same group
                        if r == 0 and hx4[g] is None:
                            hx4[g] = hsxpool.tile(
                                [128, 4, 2, 512], FP8, tag="hsx",
                                name=f"hsx{j}_{g}",
                            )
                            # j0: later hs groups ride the SWDGE queue so the
                            # HWDGE slots stay free for the weight stream
                            (nc.gpsimd if j == 0 else nc.sync).dma_start(
                                hx4[g][:], hs_h[:, 4 * g:4 * (g + 1), :, jsl]
                            )
                            if j == 0 and g == 1 and hx4[3] is None:
                                # issue g3 ahead of g2: the tail entry of the
                                # SWDGE queue lands ~5us late, and g2 has
                                # twice g3's slack to absorb that
                                hx4[3] = hsxpool.tile(
                                    [128, 4, 2, 512], FP8, tag="hsx",
                                    name="hsx0_3e",
                                )
                                nc.gpsimd.dma_start(
                                    hx4[3][:], hs_h[:, 12:16, :, jsl])
                        for hh in range(QH_L + 1):
                            w_il = wq if hh < QH_L else wk
                            hsl = (slice(hh * 128, (hh + 1) * 128)
                                   if hh < QH_L else slice(0, 128))
                            nc.tensor.matmul(
                                pp[hh][:], w_il[:, t:t + 2, 0, hsl],
                                hx4[g][:, r:r + 2, 1, :],
                                perf_mode=PM.DoubleRow,
                                start=(tp == 0), stop=False,
                            )
                            nc.tensor.matmul(
                                pp[hh][:], w_il[:, t, :, hsl],
                                hx4[g][:, r, :, :],
                                perf_mode=PM.DoubleRow,
                                start=False, stop=False,
                            )
                            nc.tensor.matmul(
                                pp[hh][:], w_il[:, t + 1, :, hsl],
                                hx4[g][:, r + 1, :, :],
                                perf_mode=PM.DoubleRow,
                                start=False, stop=(tp == NH // 2 - 1),
                            )
                        # v accumulates directly in [seq, VD] (hs-tile
                        # stationary, wv moving): no PE transpose, no
                        # staging copy
                        for st in range(4):
                            ssl = slice(st * 128, (st + 1) * 128)
                            # one start=True per PSUM bank: start zeroes at
                            # bank granularity, so the other column-groups
                            # rely on first-touch overwrite via has_written
                            nc.tensor.matmul(
                                ppv[:, ssl],
                                hx4[g][:, r:r + 2, 1, ssl],
                                wv[:, t:t + 2, 0, :],
                                perf_mode=PM.DoubleRow,
                                start=(tp == 0 and st == 0), stop=False,
                            )
                            nc.tensor.matmul(
                                ppv[:, ssl],
                                hx4[g][:, r, :, ssl],
                                wv[:, t, :, :],
                                perf_mode=PM.DoubleRow,
                                start=False, stop=False,
                            )
                            nc.tensor.matmul(
                                ppv[:, ssl],
                                hx4[g][:, r + 1, :, ssl],
                                wv[:, t + 1, :, :],
                                perf_mode=PM.DoubleRow,
                                start=False, stop=(tp == NH // 2 - 1),
                            )
                    # free the PSUM slots promptly (ACT copies). On the
                    # last strip, copy in PSUM-slot order: phase B's pools
                    # reuse these banks from bank 0 upward, so freeing low
                    # banks first lets the first scores matmul start early
                    # instead of waiting for the whole copy chain (bank-reuse
                    # WAR).
                    if j == NJ - 1:
                        order = sorted(range(QH_L + 1),
                                       key=lambda i: (pp_slot0 + i) % 6)
                    else:
                        order = list(range(QH_L + 1))
                    for ci, hh in enumerate(order):
                        tgt = qT[hh] if hh < QH_L else kT
                        if j == NJ - 1 and ci % 2 == 1:
                            # last strip: split the copy chain across ACT and
                            # DVE so the phase-A pool-close barrier (which
                            # gates phase B's PSUM pools) clears sooner
                            nc.vector.tensor_copy(tgt[:, jsl], pp[hh][:])
                        else:
                            nc.scalar.copy(tgt[:, jsl], pp[hh][:])
                    for st in range(4):
                        psl = slice(st * 128, (st + 1) * 128)
                        if j == NJ - 1 and st % 2 == 0:
                            nc.scalar.copy(vsb[:, 4 * j + st, :], ppv[:, psl])
                        else:
                            nc.vector.tensor_copy(vsb[:, 4 * j + st, :],
                                                  ppv[:, psl])
                    # cos-products for this strip's rope, early on Pool
                    m1s = []
                    for hh in range(QH_L + 1):
                        dst = qT[hh] if hh < QH_L else kT
                        m1 = spool.tile([R, 512], BF16, tag="m1", bufs=24,
                                        name=f"m1_{j}_{hh}")
                        nc.gpsimd.tensor_tensor(
                            m1[:], dst[0:R, jsl], csT[:, jsl], ALU.mult
                        )
                        m1s.append(m1)
                    # prefetch one wo column-chunk per strip on the gpsimd
                    # queue, behind this strip's weight/hs traffic
                    nc.gpsimd.dma_start(
                        wo[:, :, 512 * j:512 * (j + 1)],
                        wo_h[:, :, 512 * j:512 * (j + 1)],
                    )
                    if rope_pending is not None:
                        rope_pending()
                    # phase B runs strips in order [NJ-2, NJ-1, NJ-3, ..., 0]:
                    # only strip NJ-2 needs its q-rope by the end of phase A,
                    # and only strips <= NJ-2 need k/v; everything else
                    # defers into phase B's bubble-filler queue
                    if j == NJ - 2:
                        def rope_full(jj=j, mm=m1s):
                            for hh in range(QH_L + 1):
                                rope_head(jj, hh, mm[hh], psA, "pp")
                        rope_pending = rope_full
                    elif j < NJ - 2:
                        rope_pending = (lambda jj=j, mm=m1s:
                                        rope_head(jj, QH_L, mm[QH_L],
                                                  psA, "pp"))
                        for hh in range(QH_L):
                            rope_defer.append(
                                (j, lambda p, op=False, jj=j, h=hh,
                                 m=m1s[hh]:
                                 rope_head(jj, h, m, p, "oc", on_pool=op)))
                    else:
                        rope_pending = None
                        for hh in range(QH_L + 1):
                            rope_defer.append(
                                (j, lambda p, op=False, jj=j, h=hh,
                                 m=m1s[hh]:
                                 rope_head(jj, h, m, p, "oc", on_pool=op)))

            # ================= phases B + C (interleaved per strip) ========
            with (
                tc.tile_pool(name="accp", bufs=3) as accpool,
                tc.tile_pool(name="dnp", bufs=2) as dnpool,
                tc.tile_pool(name="osbp", bufs=6) as osbpool,
                tc.tile_pool(name="psSC", bufs=2, space="PSUM") as psSC,
                tc.tile_pool(name="psO", bufs=2, space="PSUM") as psO,
            ):
                pending = []
                _copy_alt = [0]

                def emit_tail(tail, nsplit=1):
                    acc_, oacc_, h_, j_ = tail
                    w = 512 // nsplit
                    for q in range(nsplit):
                        sl = slice(q * w, (q + 1) * w)
                        osl = slice(512 * j_ + q * w, 512 * j_ + (q + 1) * w)
                        dnb = dnpool.tile([128, w], F32, tag="dnb", bufs=4,
                                          name=f"dnb{h_}_{j_}_{q}")
                        nc.gpsimd.partition_all_reduce(
                            dnb[:], acc_[:, sl], channels=128,
                            reduce_op=bass_isa.ReduceOp.add,
                        )
                        rc = dnpool.tile([128, w], F32, tag="rc", bufs=4,
                                         name=f"rc{h_}_{j_}_{q}")
                        nc.vector.reciprocal(rc[:], dnb[:])
                        nc.vector.tensor_tensor(
                            oT[h_][:, osl], oacc_[:, sl], rc[:], ALU.mult
                        )

                def emit_group_pairs(groups, j):
                    """Emit the block streams of 2 head-groups interleaved by
                    pair index; queues their softmax tails."""
                    jsl = slice(512 * j, 512 * (j + 1))
                    state = []
                    for h in groups:
                        blocks = [(kt,) + schedule[j][kt]
                                  for kt in range(NT)
                                  if schedule[j][kt] is not None]
                        if not blocks:
                            nc.vector.memset(oT[h][:, jsl], 0.0)
                            continue
                        oacc = psO.tile([128, 512], F32, tag="oacc",
                                        name=f"oacc{h}_{j}")
                        acc = accpool.tile([128, 512], BF16, tag="acc",
                                           name=f"acc{h}_{j}")
                        state.append({
                            "h": h, "oacc": oacc, "acc": acc,
                            "pairs": _pack_pairs(blocks),
                            "first": True, "nb": len(blocks), "bi": 0,
                        })
                    npairs = max((len(st["pairs"]) for st in state), default=0)
                    for pi in range(npairs):
                        # pass 1: scores + exp for every live state, so the
                        # filler below lands exactly in the exp-wait window
                        # of the PE stream
                        for st in state:
                            if pi >= len(st["pairs"]):
                                continue
                            grp, offs, runs = st["pairs"][pi]
                            h = st["h"]
                            sc = psSC.tile([128, 1024], F32, tag="sc",
                                           name=f"sc{h}_{j}_{pi}")
                            for (kt, qlo, qhi, midx, mlo, mhi), off in zip(
                                    grp, offs):
                                w = qhi - qlo
                                nc.tensor.matmul(
                                    sc[:, off:off + w],
                                    kT[:, kt * 128:(kt + 1) * 128],
                                    qT[h][:, 512 * j + qlo:512 * j + qhi],
                                    start=True, stop=True,
                                )
                            pr = prpool.tile([128, 1024], BF16, tag="pr")
                            # exp outranks any queued filler copy on ACT:
                            # the whole pair stream hangs off its latency
                            with tc.high_priority(offset=400):
                                for lo, hi in runs:
                                    nc.scalar.activation(
                                        pr[:, lo:hi], sc[:, lo:hi], AF.Exp,
                                        scale=EXP_SCALE,
                                    )
                            st["pr"] = pr
                        if pi > 0:
                            # fill the exp-wait PE bubble with deferred rope
                            # work or o-proj chunks of a finished strip
                            if fill_queue:
                                # at most one rope thunk per slot: since the
                                # stream_shuffle rope these are pure DVE
                                # chains, and stacking two saturates the DVE
                                fill_queue.pop(0)[1](psVC)
                                if c_queue:
                                    c_queue.pop(0)()
                            else:
                                if c_queue:
                                    c_queue.pop(0)()
                        else:
                            # flush the previous group's softmax tails now:
                            # their normalizes release the oacc slots this
                            # group's first PV writes are about to need
                            while pending:
                                emit_tail(pending.pop(0), nsplit=2)
                        # pass 2: mask + PV + denominator chain
                        for st in state:
                            if pi >= len(st["pairs"]):
                                continue
                            grp, offs, runs = st["pairs"][pi]
                            h = st["h"]
                            pr = st["pr"]
                            for (kt, qlo, qhi, midx, mlo, mhi), off in zip(
                                    grp, offs):
                                w = qhi - qlo
                                prz = pr[:, off:off + w]
                                if midx >= 0:
                                    pm = pr[:, off + mlo - qlo:off + mhi - qlo]
                                    nc.vector.tensor_tensor(
                                        pm, pm, emask[midx][:, mlo:mhi],
                                        ALU.mult,
                                    )
                                last = st["bi"] == st["nb"] - 1
                                nc.tensor.matmul(
                                    st["oacc"][:, qlo:qhi],
                                    vsb[:, kt, :], prz,
                                    start=st["first"], stop=last,
                                )
                                if st["first"]:
                                    # first block spans the full strip for any
                                    # schedule reaching here via fast path;
                                    # init acc = pr + exp(sink)/128
                                    nc.vector.tensor_scalar_add(
                                        st["acc"][:, qlo:qhi], prz,
                                        sink128[:, h:h + 1],
                                    )
                                    if qlo != 0 or qhi != 512:
                                        # general-mask fallback: zero the rest
                                        if qlo > 0:
                                            nc.vector.memset(
                                                st["acc"][:, 0:qlo], 0.0)
                                        if qhi < 512:
                                            nc.vector.memset(
                                                st["acc"][:, qhi:512], 0.0)
                                else:
                                    nc.vector.tensor_tensor(
                                        st["acc"][:, qlo:qhi],
                                        st["acc"][:, qlo:qhi], prz, ALU.add,
                                    )
                                st["first"] = False
                                st["bi"] += 1
                    for st in state:
                        pending.append((st["acc"], st["oacc"], st["h"], j))

                def emit_c_chunk(qt, hc, drain=False, final=False):
                    qsl = slice(qt * 128, (qt + 1) * 128)
                    if drain and _copy_alt[0] % 3 == 1:
                        # phase B is done during the drain; borrow the idle
                        # attention-accumulator pool for extra in-flight
                        # o-proj accumulators
                        oc = psO.tile([128, 512], F32, tag="oacc",
                                      name=f"ocb{qt}_{hc}")[:]
                    elif drain and _copy_alt[0] % 3 == 2:
                        oc = psSC.tile([128, 1024], F32, tag="sc",
                                       name=f"ocs{qt}_{hc}")[:, 0:512]
                    else:
                        oc = psVC.tile([128, 512], F32, tag="oc",
                                       name=f"oc{qt}_{hc}")[:]
                    for t in range(DT):
                        nc.tensor.matmul(
                            oc, oT[t][:, qsl],
                            wo[:, t, hc * 512:(hc + 1) * 512],
                            start=(t == 0), stop=(t == DT - 1),
                        )
                    osb = osbpool.tile([128, 512], BF16, tag="osb",
                                       name=f"osb{qt}_{hc}")
                    if _copy_alt[0] % 2 == 0:
                        nc.scalar.copy(osb[:], oc)
                    else:
                        nc.vector.tensor_copy(osb[:], oc)
                    (nc.scalar if final and _copy_alt[0] % 2 == 0
                     else nc.sync).dma_start(
                        out_h[qsl, hc * 512:(hc + 1) * 512], osb[:]
                    )
                    _copy_alt[0] += 1

                # strip order: start with a dense strip (strip NJ-2) so the
                # phase boundary keeps the PE fed while deferred rope work
                # slots into its bubbles; each strip's o-projection chunks
                # interleave into later strips' pair streams as PE filler
                if NJ == 4:
                    # tuned: dense strip first (its rope is the only eager
                    # one), tiny strip 0 next while C-2 chunks fill, then the
                    # densest strip 3, then 1 ahead of the drain
                    strip_order = [2, 0, 1, 3]
                elif NJ >= 2:
                    strip_order = [NJ - 2, NJ - 1] + list(range(NJ - 2))
                else:
                    strip_order = list(range(NJ))
                border = {j: i for i, j in enumerate(strip_order)}
                fill_queue = sorted(rope_defer, key=lambda e: border[e[0]])
                c_queue = []
                for si, j in enumerate(strip_order):
                    # deferred rope for this strip must be emitted before its
                    # attention reads q/k/v
                    rest = []
                    for s, th in fill_queue:
                        if s == j:
                            th(psVC)
                        else:
                            rest.append((s, th))
                    fill_queue[:] = rest
                    for hh in range(0, QH_L, 2):
                        emit_group_pairs([hh, hh + 1], j)
                        for _ in range(4):
                            if c_queue:
                                c_queue.pop(0)()
                    while pending:
                        emit_tail(pending.pop(0), nsplit=2)
                    # queue this strip's o-projection chunks
                    for qt in range(4 * j, 4 * j + 4):
                        for hc in range(HID // 512):
                            c_queue.append(
                                lambda q=qt, c=hc, **kw:
                                emit_c_chunk(q, c, **kw))
                # drain remaining o-projection chunks; the last few spread
                # their output DMAs across queues so the epilogue flushes in
                # parallel
                for ci, thunk in enumerate(c_queue):
                    thunk(drain=True, final=(ci >= len(c_queue) - 4))
    lp.__exit__(None, None, None)
    nc.compile()
    return nc


def _classify_mask(mask):
    """Split the additive mask into 512x128 blocks (q-strip j, k-tile kt):
    fully-masked blocks are skipped; others are trimmed to the valid q-row
    range and annotated with the exp(mask) tile + its nonzero q-range."""
    schedule = [[None] * NT for _ in range(NJ)]
    tiles = []
    seen = {}
    for j in range(NJ):
        for kt in range(NT):
            blk = mask[512 * j:512 * (j + 1), 128 * kt:128 * (kt + 1)]
            valid = ~np.all(blk <= -1e8, axis=1)
            if not valid.any():
                continue
            qlo = int(valid.argmax())
            qhi = int(len(valid) - valid[::-1].argmax())
            nz = np.any(blk[qlo:qhi] != 0.0, axis=1)
            if not nz.any():
                schedule[j][kt] = (qlo, qhi, -1, 0, 0)
            else:
                mlo = qlo + int(nz.argmax())
                mhi = qhi - int(nz[::-1].argmax())
                key = blk.tobytes()
                idx = seen.get(key)
                if idx is None:
                    idx = len(tiles)
                    seen[key] = idx
                    with np.errstate(under="ignore"):
                        tiles.append(np.exp(blk.T))  # [128 k, 512 q]
                schedule[j][kt] = (qlo, qhi, idx, mlo, mhi)
    if tiles:
        em = np.ascontiguousarray(np.stack(tiles), dtype=np.float32)
    else:
        em = np.zeros((1, 128, 512), np.float32)
    return schedule, em


def _pt_layout(a, p=128):
    """[T*p, M] -> [p, T, M] (partition-major tiling along the first axis)."""
    t = a.shape[0] // p
    return np.ascontiguousarray(
        a.reshape(t, p, a.shape[1]).transpose(1, 0, 2)
    )


def _bf(a):
    return np.ascontiguousarray(a).astype(BF16_NP)


def _fp8_hilo(a, hi_first):
    """Split f32 array a (last-axis-agnostic) into (slot0, slot1) fp8 e4m3
    pair stacked on a new axis -2: hi = fp8(a), lo = fp8(a - hi)."""
    hi = np.asarray(a, dtype=np.float32).astype(FP8_NP)
    lo = (np.asarray(a, dtype=np.float32) - hi.astype(np.float32)).astype(FP8_NP)
    pair = (hi, lo) if hi_first else (lo, hi)
    return np.ascontiguousarray(np.stack(pair, axis=-2))


def _make_in_maps(inputs):
    hs = np.asarray(inputs["hidden_states"], dtype=np.float32)[0]      # [S, HID]
    cos = np.asarray(inputs["cos"], dtype=np.float32)[0]               # [S, R]
    sin = np.asarray(inputs["sin"], dtype=np.float32)[0]
    mask = np.asarray(inputs["attention_mask"], dtype=np.float32)[0, 0]
    Wq = np.asarray(inputs["Wq"], dtype=np.float32)                    # [H*D, HID]
    Wk = np.asarray(inputs["Wk"], dtype=np.float32)
    Wv = np.asarray(inputs["Wv"], dtype=np.float32)
    Wo = np.asarray(inputs["Wo"], dtype=np.float32)                    # [HID, H*VD]
    sink = np.asarray(inputs["sink_bias"], dtype=np.float32)           # [H]

    _, em = _classify_mask(mask)

    # interleaved rope-dim layout: new row 2d = orig d, 2d+1 = orig d+32,
    # applied to the q/k projection rows and the cos/sin tables. QK dot
    # products are invariant under the shared permutation, and rotate-half
    # becomes an adjacent-row swap (DVE stream_shuffle) with the sign folded
    # into the signed sin table.
    half = R // 2
    perm = np.empty(R, np.int64)
    perm[0::2] = np.arange(half)
    perm[1::2] = np.arange(half) + half
    sgn = np.where(np.arange(R) % 2 == 0, -1.0, 1.0).astype(np.float32)

    def _rope_rows(W, heads):
        Wp = W.copy()
        for hh in range(heads):
            Wp[hh * 128:hh * 128 + R] = W[hh * 128:hh * 128 + R][perm]
        return Wp

    Wq = _rope_rows(Wq, H)
    Wk = _rope_rows(Wk, KVH)

    common = {
        # [128, NH, 2, S], fp8 hi/lo with slot order (lo, hi)
        "hsx": _fp8_hilo(_pt_layout(hs.T), hi_first=False),
        "csT": _bf(cos.T[perm]),
        "snT": _bf(sin.T[perm] * sgn[:, None]),       # [64, S]
        "emask": _bf(em),
    }
    # q/k/v weights scaled by WSCALE for the fp8 e4m3 normal range; q,k
    # scale cancels in EXP_SCALE; v's scale is divided back out of Wo
    # (exact power of two in bf16)
    Wq = Wq * WSCALE
    Wk = Wk * WSCALE
    Wv = Wv * WSCALE
    Wo = Wo / WSCALE
    in_maps = []
    for i in range(N_CORES):
        se = np.exp(sink[i * QH_L:(i + 1) * QH_L]).astype(np.float32) / 128.0
        sink128 = np.broadcast_to(se[None, :], (128, QH_L))
        in_maps.append({
            **common,
            # weight slot order (hi, lo)
            "wq": _fp8_hilo(_pt_layout(np.ascontiguousarray(Wq[i * 512:(i + 1) * 512].T)), hi_first=True),
            "wk": _fp8_hilo(_pt_layout(np.ascontiguousarray(Wk[i * 128:(i + 1) * 128].T)), hi_first=True),
            "wv": _fp8_hilo(_pt_layout(np.ascontiguousarray(Wv[i * 128:(i + 1) * 128].T)), hi_first=True),
            "wo": _bf(_pt_layout(np.ascontiguousarray(Wo[:, i * 512:(i + 1) * 512].T))),
            "sink128": np.ascontiguousarray(sink128, dtype=np.float32),
        })
    return in_maps


def _schedule_key(schedule):
    return tuple(tuple(r) for r in schedule)


def kernel(**inputs):
    mask = np.asarray(inputs["attention_mask"], dtype=np.float32)[0, 0]
    schedule, em = _classify_mask(mask)
    key = _schedule_key(schedule)
    if key not in _cache:
        _cache[key] = _build(schedule, em.shape[0])
    nc = _cache[key]

    in_maps = _make_in_maps(inputs)

    res = run_bass_kernel_spmd(nc, in_maps, list(range(N_CORES)))
    out = np.zeros((S, HID), np.float64)
    for i in range(N_CORES):
        out += np.asarray(res.results[i]["out"], dtype=np.float64)
    out = out.astype(np.float32).reshape(B, S, HID)
    if not np.isfinite(out).all():
        raise FloatingPointError(
            "kernel produced non-finite values (softmax logits exceeded the "
            "no-max-pass exp range); inputs are outside the validated regime"
        )
    return out

